# revision 11
# baseline (speedup 1.0000x reference)
"""Trainium2 Bass kernel for nn_BoxDetectionLoss (8-core data parallel).

Math: reference loss = sum_{a,r,c}[ has_match ? coord+conf_loss : conf^2 ] / denom.
A pixel (r,c) can only match a target box t if r==tb[t,0] and c==tb[t,1]
(T=16 boxes per image), so the dense term is just sum sigmoid(conf_ch)^2 over
channels {2,5,8}; the match term is a correction at <=16 pixels x 3 anchors,
computed from 144 gathered elements per image.

Each of the 8 cores handles one batch image.  v2 layout:
  - dense conf channels stream in NCHUNK-per-channel chunks, alternating
    between the sync HWDGE ring and the gpsimd SWDGE ring so both DMA rings
    pull concurrently; ACT runs sigmoid per chunk as it lands and DVE does a
    fused square+reduce (tensor_tensor_reduce) per chunk into one ACC column.
  - all tiny correction constants (tb/tp/tri/choff/TBrep, pre-converted to
    f32 on host) arrive in ONE [16,94] DMA on the sync ring ahead of the
    dense chunks; the correction chain (gather offsets, indirect gather,
    pred/match/dup math) runs on DVE in the shadow of the dense stream.
  - final: DVE reduces ACC -> [128,1], DMA to HBM; host sums 128x8 partials.
"""

import numpy as np

B, C, H, W = 8, 9, 512, 512
T = 16
N_CORES = 8
CONF_CH = (2, 5, 8)
DENOM = float(B * H * W * 3)
MAGIC = 12582912.0  # 1.5 * 2^23: x+MAGIC-MAGIC rounds to nearest-even int

import os
DENSE_MODE = os.environ.get("DENSE_MODE", "sp_gp")
SQ_MODE = os.environ.get("SQ_MODE", "ttr")      # ttr | tt_red
OUT_MODE = os.environ.get("OUT_MODE", "vec")    # vec | pe
CORR = os.environ.get("CORR", "1") == "1"

NCHUNK = 4                      # chunks per channel
NDENSE = len(CONF_CH) * NCHUNK  # dense ACC columns
CCOL = NDENSE                   # correction column
CHUNK_COLS = 2048 // NCHUNK

# packed constant block: [T, 94] f32
#   0:4   tb (as float)
#   4     tp
#   5:21  tri (strictly-lower mask)
#   21:30 choff (ch*H*W)
#   30:94 TBrep (tb flattened, broadcast to all rows)
CST_COLS = 94


def make_cst(tb_i, tp_i):
    cst = np.zeros((T, CST_COLS), dtype=np.float32)
    tbf = tb_i.astype(np.float32)
    cst[:, 0:4] = tbf
    cst[:, 4] = tp_i
    cst[:, 5:21] = np.tril(np.ones((T, T), dtype=np.float32), -1)
    cst[:, 21:30] = (np.arange(C, dtype=np.float32) * (H * W))[None, :]
    cst[:, 30:94] = tbf.reshape(1, 4 * T)
    return cst


_PROG = None


def _build_correction_a(nc, sp, bass, mybir, CST, pol, gather=True):
    f32 = mybir.dt.float32
    i32 = mybir.dt.int32
    ALU = mybir.AluOpType

    TBf = CST[:, 0:4]
    TP = CST[:, 4:5]
    TRI = CST[:, 5:21]
    CH = CST[:, 21:30]
    rep4 = CST[:, 30:94].rearrange("p (t f) -> p f t", f=4)

    # packed coords: p1 = r*512 + c, p2 = r2*512 + c2 (exact in f32)
    p1 = sp.tile([T, 1], f32)
    nc.vector.tensor_scalar(
        out=p1[:], in0=TBf[:, 0:1], scalar1=512.0, scalar2=TBf[:, 1:2],
        op0=ALU.mult, op1=ALU.add,
    )
    p2 = sp.tile([T, 1], f32)
    nc.vector.tensor_scalar(
        out=p2[:], in0=TBf[:, 2:3], scalar1=512.0, scalar2=TBf[:, 3:4],
        op0=ALU.mult, op1=ALU.add,
    )

    # row-layout packed coords of all boxes, from the replicated copy
    p1row = sp.tile([T, T], f32)
    nc.vector.tensor_scalar(
        out=p1row[:], in0=rep4[:, 0, :], scalar1=512.0, scalar2=None,
        op0=ALU.mult,
    )
    nc.vector.tensor_tensor(
        out=p1row[:], in0=p1row[:], in1=rep4[:, 1, :], op=ALU.add
    )
    p2row = sp.tile([T, T], f32)
    nc.vector.tensor_scalar(
        out=p2row[:], in0=rep4[:, 2, :], scalar1=512.0, scalar2=None,
        op0=ALU.mult,
    )
    nc.vector.tensor_tensor(
        out=p2row[:], in0=p2row[:], in1=rep4[:, 3, :], op=ALU.add
    )

    # duplicate-box detection: S[t,t'] = (p1 equal) & (p2 equal), t' < t
    S = sp.tile([T, T], f32)
    nc.vector.tensor_scalar(
        out=S[:], in0=p1row[:], scalar1=p1[:], scalar2=None, op0=ALU.is_equal
    )
    S2 = sp.tile([T, T], f32)
    nc.vector.tensor_scalar(
        out=S2[:], in0=p2row[:], scalar1=p2[:], scalar2=None, op0=ALU.is_equal
    )
    nc.vector.tensor_tensor(out=S[:], in0=S[:], in1=S2[:], op=ALU.mult)
    nc.vector.tensor_tensor(out=S[:], in0=S[:], in1=TRI[:], op=ALU.mult)
    dupc = sp.tile([T, 1], f32)
    nc.vector.tensor_reduce(
        out=dupc[:], in_=S[:], axis=mybir.AxisListType.X, op=ALU.add
    )
    keep = sp.tile([T, 1], f32)
    nc.vector.tensor_scalar(
        out=keep[:], in0=dupc[:], scalar1=0.0, scalar2=None, op0=ALU.is_equal
    )

    # gather pol[ch, tb0[t], tb1[t]] for all (t, ch): offsets = ch*H*W + p1
    OFFf = sp.tile([T, C], f32)
    nc.vector.tensor_scalar(
        out=OFFf[:], in0=CH[:], scalar1=p1[:], scalar2=None, op0=ALU.add
    )
    OFFi = sp.tile([T, C], i32)
    nc.vector.tensor_copy(OFFi[:], OFFf[:])
    G = sp.tile([T, C], f32)
    if gather:
        nc.gpsimd.indirect_dma_start(
            out=G[:], out_offset=None,
            in_=pol.rearrange("c h (w a) -> (c h w) a", a=1),
            in_offset=bass.IndirectOffsetOnAxis(ap=OFFi[:], axis=0),
        )
    else:
        nc.vector.memset(G[:], 0.0)
    return dict(TBf=TBf, TP=TP, keep=keep, G=G)


def _build_correction_b(nc, sp, ACC, bass, mybir, ctx):
    f32 = mybir.dt.float32
    ALU = mybir.AluOpType
    ACT_F = mybir.ActivationFunctionType
    TP, TBf, keep, G = ctx["TP"], ctx["TBf"], ctx["keep"], ctx["G"]

    GS = sp.tile([T, C], f32)
    nc.scalar.activation(GS[:], G[:], ACT_F.Sigmoid)
    # channel ch = 3a + k: k=0 delta_r, k=1 delta_c, k=2 conf
    gs3 = GS[:].rearrange("p (a k) -> p k a", k=3)

    # pred = clip(tb + sigmoid*scale, 0, 511), all 3 anchors at once
    predr = sp.tile([T, 3], f32)
    nc.vector.tensor_scalar(
        out=predr[:], in0=gs3[:, 0, :], scalar1=9.0, scalar2=TBf[:, 0:1],
        op0=ALU.mult, op1=ALU.add,
    )
    nc.vector.tensor_scalar(
        out=predr[:], in0=predr[:], scalar1=511.0, scalar2=0.0,
        op0=ALU.min, op1=ALU.max,
    )
    predc = sp.tile([T, 3], f32)
    nc.vector.tensor_scalar(
        out=predc[:], in0=gs3[:, 1, :], scalar1=16.0, scalar2=TBf[:, 1:2],
        op0=ALU.mult, op1=ALU.add,
    )
    nc.vector.tensor_scalar(
        out=predc[:], in0=predc[:], scalar1=511.0, scalar2=0.0,
        op0=ALU.min, op1=ALU.max,
    )

    # round to nearest-even integer: (x + 1.5*2^23) - 1.5*2^23
    rr = sp.tile([T, 3], f32)
    nc.vector.tensor_scalar(
        out=rr[:], in0=predr[:], scalar1=MAGIC, scalar2=None, op0=ALU.add
    )
    nc.vector.tensor_scalar(
        out=rr[:], in0=rr[:], scalar1=MAGIC, scalar2=None, op0=ALU.subtract
    )
    rc = sp.tile([T, 3], f32)
    nc.vector.tensor_scalar(
        out=rc[:], in0=predc[:], scalar1=MAGIC, scalar2=None, op0=ALU.add
    )
    nc.vector.tensor_scalar(
        out=rc[:], in0=rc[:], scalar1=MAGIC, scalar2=None, op0=ALU.subtract
    )

    # match mask per (t, anchor)
    m = sp.tile([T, 3], f32)
    nc.vector.tensor_scalar(
        out=m[:], in0=rr[:], scalar1=TBf[:, 2:3], scalar2=None, op0=ALU.is_equal
    )
    m2 = sp.tile([T, 3], f32)
    nc.vector.tensor_scalar(
        out=m2[:], in0=rc[:], scalar1=TBf[:, 3:4], scalar2=None, op0=ALU.is_equal
    )
    nc.vector.tensor_tensor(out=m[:], in0=m[:], in1=m2[:], op=ALU.mult)

    # contribution = |predr-tb2| + |predc-tb3| + tp*(tp-2*conf)
    # |x| as max(x, -x) on DVE - keeps Abs out of the ACT function table
    d1 = sp.tile([T, 3], f32)
    nc.vector.tensor_scalar(
        out=d1[:], in0=predr[:], scalar1=TBf[:, 2:3], scalar2=None,
        op0=ALU.subtract,
    )
    d1n = sp.tile([T, 3], f32)
    nc.vector.tensor_scalar(
        out=d1n[:], in0=d1[:], scalar1=-1.0, scalar2=None, op0=ALU.mult
    )
    nc.vector.tensor_tensor(out=d1[:], in0=d1[:], in1=d1n[:], op=ALU.max)
    d2 = sp.tile([T, 3], f32)
    nc.vector.tensor_scalar(
        out=d2[:], in0=predc[:], scalar1=TBf[:, 3:4], scalar2=None,
        op0=ALU.subtract,
    )
    d2n = sp.tile([T, 3], f32)
    nc.vector.tensor_scalar(
        out=d2n[:], in0=d2[:], scalar1=-1.0, scalar2=None, op0=ALU.mult
    )
    nc.vector.tensor_tensor(out=d2[:], in0=d2[:], in1=d2n[:], op=ALU.max)
    nc.vector.tensor_tensor(out=d1[:], in0=d1[:], in1=d2[:], op=ALU.add)
    cf = sp.tile([T, 3], f32)
    nc.vector.tensor_scalar(
        out=cf[:], in0=gs3[:, 2, :], scalar1=-2.0, scalar2=TP[:],
        op0=ALU.mult, op1=ALU.add,
    )
    nc.vector.tensor_scalar(
        out=cf[:], in0=cf[:], scalar1=TP[:], scalar2=None, op0=ALU.mult
    )
    nc.vector.tensor_tensor(out=d1[:], in0=d1[:], in1=cf[:], op=ALU.add)
    # valid = match * keep; corr contribution = valid * d1
    nc.vector.tensor_scalar(
        out=m[:], in0=m[:], scalar1=keep[:], scalar2=None, op0=ALU.mult
    )
    nc.vector.tensor_tensor(out=m[:], in0=m[:], in1=d1[:], op=ALU.mult)
    nc.vector.tensor_reduce(
        out=ACC[0:T, CCOL : CCOL + 1], in_=m[:],
        axis=mybir.AxisListType.X, op=ALU.add,
    )


def _build_program(corr=CORR, gather=True, dense_mode=DENSE_MODE,
                   corr_after=6):
    import concourse.bass as bass
    import concourse.tile as tile
    from concourse import bacc, mybir

    f32 = mybir.dt.float32
    ALU = mybir.AluOpType
    ACT_F = mybir.ActivationFunctionType

    nc = bacc.Bacc(
        "TRN2", target_bir_lowering=False, debug=False, num_devices=N_CORES
    )
    pol = nc.dram_tensor("pol", [C, H, W], f32, kind="ExternalInput").ap()
    cst = nc.dram_tensor("cst", [T, CST_COLS], f32, kind="ExternalInput").ap()
    out = nc.dram_tensor(
        "out", [128] if OUT_MODE == "vec" else [1], f32, kind="ExternalOutput"
    ).ap()

    with tile.TileContext(nc) as tc:
        with (
            tc.tile_pool(name="io", bufs=1) as io,
            tc.tile_pool(name="acc", bufs=1) as accp,
            tc.tile_pool(name="small", bufs=1) as sp,
        ):
            ACC = accp.tile([128, NDENSE + 1], f32)

            # ---------- constants first on the sync ring (tiny) ----------
            CSTt = sp.tile([T, CST_COLS], f32)
            nc.sync.dma_start(CSTt[:], cst[:])

            # ---------- dense chunk DMAs: even chunks on sync HWDGE,
            # odd chunks on gpsimd SWDGE (second ring) ----------
            views = [
                pol[ch].rearrange("(p a) w -> p (a w)", p=128) for ch in CONF_CH
            ]
            nchunks = len(CONF_CH) * NCHUNK
            ctiles = []
            for ci in range(nchunks):
                ctiles.append(
                    io.tile([128, CHUNK_COLS], f32, name=f"in{ci}", tag=f"in{ci}")
                )

            def issue(ci, engine):
                ch, k = divmod(ci, NCHUNK)
                cols = slice(k * CHUNK_COLS, (k + 1) * CHUNK_COLS)
                engine.dma_start(ctiles[ci][:], views[ch][:, cols])

            if dense_mode == "sp_gp":
                even_eng, odd_eng = nc.sync, nc.gpsimd
            elif dense_mode == "sp_act":
                even_eng, odd_eng = nc.sync, nc.scalar
            else:  # all on sync
                even_eng, odd_eng = nc.sync, nc.sync

            # interleave issues in chunk order so arrival matches use order;
            # odd-ring issues lead slightly (SWDGE setup is slower)
            for ci in range(nchunks):
                issue(ci, even_eng if ci % 2 == 0 else odd_eng)

            # memset correction column (corr only writes partitions 0..T-1)
            nc.vector.memset(ACC[:, CCOL : CCOL + 1], 0.0)

            if corr:
                corr_ctx = _build_correction_a(
                    nc, sp, bass, mybir, CSTt, pol, gather=gather
                )

            # ---------- dense compute: per chunk, ACT sigmoid then DVE
            # fused square+reduce into ACC[:, ci] ----------
            sigs = []
            for ci in range(nchunks):
                s = io.tile([128, CHUNK_COLS], f32, name=f"sig{ci}", tag=f"sig{ci}")
                sigs.append(s)

            def dve_square(ci):
                if SQ_MODE == "ttr":
                    nc.vector.tensor_tensor_reduce(
                        out=ctiles[ci][:], in0=sigs[ci][:], in1=sigs[ci][:],
                        scale=1.0, scalar=0.0, op0=ALU.mult, op1=ALU.add,
                        accum_out=ACC[:, ci : ci + 1],
                    )
                else:
                    nc.vector.tensor_tensor(
                        out=ctiles[ci][:], in0=sigs[ci][:], in1=sigs[ci][:],
                        op=ALU.mult,
                    )
                    nc.vector.tensor_reduce(
                        out=ACC[:, ci : ci + 1], in_=ctiles[ci][:],
                        axis=mybir.AxisListType.X, op=ALU.add,
                    )

            corr_done = not corr
            for ci in range(nchunks):
                nc.scalar.activation(sigs[ci][:], ctiles[ci][:], ACT_F.Sigmoid)
                dve_square(ci)
                if ci + 1 == corr_after and not corr_done:
                    _build_correction_b(nc, sp, ACC, bass, mybir, corr_ctx)
                    corr_done = True
            if not corr_done:
                _build_correction_b(nc, sp, ACC, bass, mybir, corr_ctx)

            RED = sp.tile([128, 1], f32)
            nc.vector.tensor_reduce(
                out=RED[:], in_=ACC[:], axis=mybir.AxisListType.X, op=ALU.add
            )
            if OUT_MODE == "vec":
                nc.sync.dma_start(out[:], RED[:])
            else:
                ONES = sp.tile([128, 1], f32)
                nc.vector.memset(ONES[:], 1.0)
                with tc.tile_pool(name="psum", bufs=1, space="PSUM") as psum:
                    PS = psum.tile([1, 1], f32, space="PSUM")
                    nc.tensor.matmul(out=PS[:], lhsT=RED[:], rhs=ONES[:],
                                     start=True, stop=True)
                    OUTSB = sp.tile([1, 1], f32)
                    nc.vector.tensor_copy(OUTSB[:], PS[:])
                    nc.scalar.dma_start(out[:], OUTSB[:])

    nc.compile()
    return nc


def get_program():
    global _PROG
    if _PROG is None:
        _PROG = _build_program()
    return _PROG


def make_in_maps(policy_output, target_boxes, target_probs):
    policy_output = np.ascontiguousarray(np.asarray(policy_output, dtype=np.float32))
    target_boxes = np.ascontiguousarray(np.asarray(target_boxes, dtype=np.int32))
    target_probs = np.ascontiguousarray(np.asarray(target_probs, dtype=np.float32))
    assert policy_output.shape == (B, C, H, W)
    in_maps = []
    for i in range(N_CORES):
        in_maps.append(
            {
                "pol": policy_output[i],
                "cst": make_cst(target_boxes[i], target_probs[i]),
            }
        )
    return in_maps


def kernel(policy_output, target_boxes, target_probs):
    from concourse.bass_utils import run_bass_kernel_spmd

    nc = get_program()
    in_maps = make_in_maps(policy_output, target_boxes, target_probs)
    res = run_bass_kernel_spmd(nc, in_maps, list(range(N_CORES)))
    total = 0.0
    for i in range(N_CORES):
        total += float(res.results[i]["out"].sum(dtype=np.float64))
    return np.float32(total / DENOM)
